# Initial kernel scaffold
#
"""Trainium2 Bass kernel for nn_CustomMultiHeadAttention_79860621902019.

Math (derived from the reference):
  key/value are broadcast along KV before the k/v projections, so
  K[b,k,:] == key[b] @ k_weight.T  for every k (same for V).  Hence
  scores[b,h,q,k] is constant along k, the softmax cancels it, and

    lin[b,h,q]  = sum(w1) * s[b,h,q] + clip[b]@w2 + scale_b
    s[b,h,q]    = dot(query[b,q,head h], Kvec[b,head h]) / sqrt(HD)
    attn[b,h,q,k] = softmax_k( lin[b,h,q] * clip[b,k] )
    out[b,q,:]  = Vvec[b,:]            (since rows of attn sum to 1)

  with Kvec = key @ k_weight.T, Vvec = value @ v_weight.T.

Sharding: data-parallel over batch B=8 across the 8 NeuronCores (one
batch element per core); the small weights are replicated.
"""

import os
import sys
import threading

if "/opt/trn_rl_repo" not in sys.path:
    sys.path.insert(0, "/opt/trn_rl_repo")

import numpy as np

B, QL, D, H, KV, HD = 8, 2048, 1024, 16, 256, 64
N_CORES = 8
NQT = QL // 128  # 16 query tiles per core
DT = None  # set after mybir import

_lock = threading.Lock()
_cache = {}


def _build_nc(repeat: int = 1):
    """Build + compile the per-core Bass program.

    repeat > 1 wraps the whole computation in a hardware For_i loop that
    redoes identical work (same addresses) -- used only for timing.
    """
    import concourse.bass as bass
    import concourse.mybir as mybir
    import concourse.tile as tile
    from concourse import bacc
    from concourse.masks import make_identity

    dt = mybir.dt.float32
    fp32 = mybir.dt.float32

    nc = bacc.Bacc("TRN2", target_bir_lowering=False, debug=False,
                   num_devices=N_CORES)

    # ---- DRAM I/O (per-core shapes) ----
    q_d = nc.dram_tensor("query", [QL, D], dt, kind="ExternalInput").ap()
    key_d = nc.dram_tensor("key", [D], dt, kind="ExternalInput").ap()
    val_d = nc.dram_tensor("value", [D], dt, kind="ExternalInput").ap()
    clip_d = nc.dram_tensor("clip_score", [KV], dt, kind="ExternalInput").ap()
    kw_d = nc.dram_tensor("k_weight", [D, D], dt, kind="ExternalInput").ap()
    vw_d = nc.dram_tensor("v_weight", [D, D], dt, kind="ExternalInput").ap()
    sw_d = nc.dram_tensor("scale_w", [2 * KV], dt, kind="ExternalInput").ap()
    sb_d = nc.dram_tensor("scale_b", [1], dt, kind="ExternalInput").ap()

    attn_d = nc.dram_tensor("attn", [H, QL, KV], dt, kind="ExternalOutput").ap()
    out_d = nc.dram_tensor("out", [QL, D], dt, kind="ExternalOutput").ap()

    # DRAM scratch for partition-broadcast bounces
    kvrow_d = nc.dram_tensor("kvec_row", [D], dt).ap()
    vvrow_d = nc.dram_tensor("vvec_row", [D], dt).ap()
    cr_d = nc.dram_tensor("consts_row", [4], dt).ap()

    def bcast(dram_ap, parts):
        """DRAM AP replicated across `parts` partitions (partition step 0)."""
        return bass.AP(tensor=dram_ap.tensor, offset=dram_ap.offset,
                       ap=[[0, parts]] + list(dram_ap.ap))

    A = mybir.AluOpType

    with tile.TileContext(nc) as tc:
        with (
            tc.tile_pool(name="const", bufs=1) as constp,
            tc.tile_pool(name="wload", bufs=3) as wloadp,
            tc.tile_pool(name="qload", bufs=3) as qloadp,
            tc.tile_pool(name="mid", bufs=3) as midp,
            tc.tile_pool(name="ebuf", bufs=2) as ebufp,
            tc.tile_pool(name="abuf", bufs=3) as abufp,
            tc.tile_pool(name="psum", bufs=2, space="PSUM") as psump,
            tc.tile_pool(name="psumc", bufs=1, space="PSUM") as psumcp,
        ):
            # ================= setup (tiny) =================
            ident = constp.tile([128, 128], dt)
            make_identity(nc, ident[:, :])

            clip_row = constp.tile([1, KV], dt)
            nc.sync.dma_start(out=clip_row[:, :],
                              in_=clip_d.rearrange("(a k) -> a k", a=1))
            clipB = constp.tile([128, KV], dt)
            nc.gpsimd.dma_start(out=clipB[:, :], in_=bcast(clip_d, 128))
            # ACT reads from PSUM are cheaper (172 vs 224 cyc fixed)
            clipB_ps = psumcp.tile([128, KV], fp32)
            nc.vector.tensor_copy(clipB_ps[:, :], clipB[:, :])

            keyB = constp.tile([128, D], dt)
            nc.gpsimd.dma_start(out=keyB[:, :], in_=bcast(key_d, 128))
            valB = constp.tile([128, D], dt)
            nc.gpsimd.dma_start(out=valB[:, :], in_=bcast(val_d, 128))

            sw_row = constp.tile([1, 2 * KV], dt)
            nc.sync.dma_start(out=sw_row[:, :],
                              in_=sw_d.rearrange("(a k) -> a k", a=1))
            sb_t = constp.tile([1, 1], dt)
            nc.sync.dma_start(out=sb_t[:, :],
                              in_=sb_d.rearrange("(a k) -> a k", a=1))

            maxc = constp.tile([1, 1], dt)
            nc.vector.tensor_reduce(maxc[:, :], clip_row[:, :],
                                    axis=mybir.AxisListType.X, op=A.max)
            minc = constp.tile([1, 1], dt)
            nc.vector.tensor_reduce(minc[:, :], clip_row[:, :],
                                    axis=mybir.AxisListType.X, op=A.min)
            cbraw = constp.tile([1, 1], dt)
            scr1 = constp.tile([1, KV], dt)
            nc.vector.tensor_tensor_reduce(
                out=scr1[:, :], in0=clip_row[:, :], in1=sw_row[:, KV:2 * KV],
                scale=1.0, scalar=0.0, op0=A.mult, op1=A.add,
                accum_out=cbraw[:, :])
            sumw1 = constp.tile([1, 1], dt)
            nc.vector.reduce_sum(sumw1[:, :], sw_row[:, 0:KV],
                                 axis=mybir.AxisListType.X)

            # consts row: [cbtot, -maxclip, -minclip, sumw1/sqrt(HD)]
            cr = constp.tile([1, 4], dt)
            nc.vector.tensor_add(cr[:, 0:1], cbraw[:, :], sb_t[:, :])
            nc.vector.tensor_scalar_mul(cr[:, 1:2], maxc[:, :], -1.0)
            nc.vector.tensor_scalar_mul(cr[:, 2:3], minc[:, :], -1.0)
            nc.vector.tensor_scalar_mul(cr[:, 3:4], sumw1[:, :], 1.0 / 8.0)
            nc.sync.dma_start(out=cr_d.rearrange("(a k) -> a k", a=1),
                              in_=cr[:, :])
            constsB = constp.tile([128, 4], dt)
            nc.gpsimd.dma_start(out=constsB[:, :], in_=bcast(cr_d, 128))
            cbtot_v = constsB[:, 0:1]
            negmaxc_v = constsB[:, 1:2]
            negminc_v = constsB[:, 2:3]
            s18_v = constsB[:, 3:4]

            # ================= Kvec / Vvec =================
            kvec_c = constp.tile([128, 8], dt)   # [d%128, d//128]
            vvec_c = constp.tile([128, 8], dt)
            for w_ap, srcB, dest in ((kw_d, keyB, kvec_c), (vw_d, valB, vvec_c)):
                for j in range(8):
                    wchunk = wloadp.tile([128, D], dt, tag="wchunk")
                    nc.sync.dma_start(out=wchunk[:, :],
                                      in_=w_ap[j * 128:(j + 1) * 128, :])
                    wscr = wloadp.tile([128, D], dt, tag="wscr")
                    nc.vector.tensor_tensor_reduce(
                        out=wscr[:, :], in0=wchunk[:, :], in1=srcB[:, :],
                        scale=1.0, scalar=0.0, op0=A.mult, op1=A.add,
                        accum_out=dest[:, j:j + 1])

            # scale Kvec by sumw1/sqrt(HD) so q.KvecB directly gives lin-minus-bias
            kvec_s = constp.tile([128, 8], dt)
            nc.vector.tensor_scalar_mul(kvec_s[:, :], kvec_c[:, :], s18_v)

            # transpose [128,8] -> [8,128] rows, bounce via DRAM, broadcast
            KvecB = constp.tile([128, D], dt)
            VvecB = constp.tile([128, D], dt)
            for src, row_d, dstB in ((kvec_s, kvrow_d, KvecB),
                                     (vvec_c, vvrow_d, VvecB)):
                ps_t = psump.tile([8, 128], fp32, tag="pst")
                nc.tensor.transpose(ps_t[:, :], src[:, :], ident[:, :])
                rows = constp.tile([8, 128], dt)
                nc.vector.tensor_copy(rows[:, :], ps_t[:, :])
                nc.sync.dma_start(out=row_d.rearrange("(a k) -> a k", a=8),
                                  in_=rows[:, :])
                nc.gpsimd.dma_start(out=dstB[:, :], in_=bcast(row_d, 128))

            # ================= main loop =================
            def body(_iv=None):
                for qt in range(NQT):
                    q0 = qt * 128
                    qtile = qloadp.tile([128, D], dt, tag="qtile")
                    nc.sync.dma_start(out=qtile[:, :],
                                      in_=q_d[q0:q0 + 128, :])

                    # out rows are just Vvec
                    nc.sync.dma_start(out=out_d[q0:q0 + 128, :],
                                      in_=VvecB[:, :])

                    prod = qloadp.tile([128, D], dt, tag="prod")
                    nc.vector.tensor_mul(prod[:, :], qtile[:, :], KvecB[:, :])
                    lin = midp.tile([128, H], dt, tag="lin")
                    nc.vector.reduce_sum(
                        lin[:, :],
                        prod[:, :].rearrange("p (h d) -> p h d", d=HD),
                        axis=mybir.AxisListType.X)
                    # lin += cbtot
                    nc.vector.tensor_scalar_add(lin[:, :], lin[:, :], cbtot_v)
                    # negm = min(lin*-maxc, lin*-minc) = -max_k(lin*clip)
                    nega = midp.tile([128, H], dt, tag="nega")
                    nc.vector.tensor_scalar_mul(nega[:, :], lin[:, :], negmaxc_v)
                    negm = midp.tile([128, H], dt, tag="negm")
                    nc.vector.tensor_scalar_mul(negm[:, :], lin[:, :], negminc_v)
                    nc.vector.tensor_tensor(negm[:, :], nega[:, :], negm[:, :],
                                            op=A.min)

                    e3 = ebufp.tile([128, H * KV], dt, tag="e3")
                    S = midp.tile([128, H], dt, tag="S")
                    for h in range(H):
                        nc.scalar.activation(
                            e3[:, h * KV:(h + 1) * KV], clipB_ps[:, :],
                            mybir.ActivationFunctionType.Exp,
                            bias=negm[:, h:h + 1], scale=lin[:, h:h + 1],
                            accum_out=S[:, h:h + 1])
                    rinv = midp.tile([128, H], dt, tag="rinv")
                    nc.vector.reciprocal(rinv[:, :], S[:, :])

                    at = abufp.tile([128, H * KV], dt, tag="at")
                    for h in range(H):
                        nc.vector.tensor_scalar_mul(
                            at[:, h * KV:(h + 1) * KV],
                            e3[:, h * KV:(h + 1) * KV], rinv[:, h:h + 1])

                    # one 2 MiB DMA: [128q, 16h, 256k] -> attn[h, q0+q, k]
                    a_sl = attn_d[:, q0:q0 + 128, :]
                    a_perm = bass.AP(tensor=a_sl.tensor, offset=a_sl.offset,
                                     ap=[list(a_sl.ap[1]), list(a_sl.ap[0]),
                                         list(a_sl.ap[2])])
                    nc.sync.dma_start(
                        out=a_perm,
                        in_=at[:, :].rearrange("p (h k) -> p h k", k=KV))

            if repeat == 1:
                body()
            else:
                with tc.For_i(0, repeat, 1) as iv:
                    body(iv)

    nc.compile()
    return nc


def _get_nc(repeat: int = 1):
    with _lock:
        key = ("nc", repeat)
        if key not in _cache:
            _cache[key] = _build_nc(repeat)
        return _cache[key]


def kernel(query, key, value, clip_score, k_weight, v_weight, scale_w,
           scale_b):
    from concourse.bass_utils import run_bass_kernel_spmd

    query = np.ascontiguousarray(np.asarray(query, dtype=np.float32))
    key = np.ascontiguousarray(np.asarray(key, dtype=np.float32))
    value = np.ascontiguousarray(np.asarray(value, dtype=np.float32))
    clip_score = np.ascontiguousarray(np.asarray(clip_score, dtype=np.float32))
    k_weight = np.ascontiguousarray(np.asarray(k_weight, dtype=np.float32))
    v_weight = np.ascontiguousarray(np.asarray(v_weight, dtype=np.float32))
    scale_w = np.ascontiguousarray(np.asarray(scale_w, dtype=np.float32))
    scale_b = np.asarray(scale_b, dtype=np.float32).reshape(1)

    nc = _get_nc()

    in_maps = []
    for b in range(B):
        in_maps.append({
            "query": query[b],
            "key": key[b],
            "value": value[b],
            "clip_score": clip_score[b],
            "k_weight": k_weight,
            "v_weight": v_weight,
            "scale_w": scale_w,
            "scale_b": scale_b,
        })

    res = run_bass_kernel_spmd(nc, in_maps, core_ids=list(range(N_CORES)))
    attn = np.stack([res.results[b]["attn"] for b in range(B)])
    out = np.stack([res.results[b]["out"] for b in range(B)])
    return out, attn


if __name__ == "__main__":
    # quick smoke: random inputs, shape check only
    rng = np.random.default_rng(0)
    o, a = kernel(
        rng.standard_normal((B, QL, D), dtype=np.float32),
        rng.standard_normal((B, D), dtype=np.float32),
        rng.standard_normal((B, D), dtype=np.float32),
        rng.random((B, KV), dtype=np.float32),
        (rng.standard_normal((D, D), dtype=np.float32) * 0.02),
        (rng.standard_normal((D, D), dtype=np.float32) * 0.02),
        np.full((2 * KV,), 0.05, dtype=np.float32),
        np.float32(7.5),
    )
    print(o.shape, a.shape)


# revision 14
# speedup vs baseline: 1.1290x; 1.1290x over previous
"""Trainium2 Bass kernel for nn_CustomMultiHeadAttention_79860621902019.

Math (derived from the reference):
  key/value are broadcast along KV before the k/v projections, so
  K[b,k,:] == key[b] @ k_weight.T  for every k (same for V).  Hence
  scores[b,h,q,k] is constant along k, the softmax cancels it, and

    lin[b,h,q]  = sum(w1) * s[b,h,q] + clip[b]@w2 + scale_b
    s[b,h,q]    = dot(query[b,q,head h], Kvec[b,head h]) / sqrt(HD)
    attn[b,h,q,k] = softmax_k( lin[b,h,q] * clip[b,k] )
    out[b,q,:]  = Vvec[b,:]            (since rows of attn sum to 1)

  with Kvec = key @ k_weight.T, Vvec = value @ v_weight.T.

Sharding: data-parallel over batch B=8 across the 8 NeuronCores (one
batch element per core); the small weights are replicated.
"""

import os
import sys
import threading

if "/opt/trn_rl_repo" not in sys.path:
    sys.path.insert(0, "/opt/trn_rl_repo")

import numpy as np

B, QL, D, H, KV, HD = 8, 2048, 1024, 16, 256, 64
N_CORES = 8
NQT = QL // 128  # 16 query tiles per core
# Shard the k/v projection contraction dim across cores (each core reads a
# 128-column slice of both weights, computes partial Kvec/Vvec for all 8
# batches on the PE, and a ReduceScatter hands each core its own batch's
# full Kvec/Vvec). Cuts per-core weight DMA 8.4 MB -> 1.05 MB and removes
# the DVE mul+reduce projection work.
WEIGHT_SHARD = True

_lock = threading.Lock()
_cache = {}


def _build_nc(repeat: int = 1, full_unroll: int = 1):
    """Build + compile the per-core Bass program.

    repeat > 1 wraps the main q-tile loop in a hardware For_i redoing
    identical work; full_unroll > 1 python-unrolls the ENTIRE program
    (setup + collective + main loop) that many times. Both are used only
    for timing (the slope vs count cancels dispatch overhead).
    """
    assert repeat == 1 or full_unroll == 1
    import concourse.bass as bass
    import concourse.mybir as mybir
    import concourse.tile as tile
    from concourse import bacc
    from concourse.masks import make_identity

    dt = mybir.dt.float32
    fp32 = mybir.dt.float32

    nc = bacc.Bacc("TRN2", target_bir_lowering=False, debug=False,
                   num_devices=N_CORES)

    # ---- DRAM I/O (per-core shapes) ----
    q_d = nc.dram_tensor("query", [QL, D], dt, kind="ExternalInput").ap()
    clip_d = nc.dram_tensor("clip_score", [KV], dt, kind="ExternalInput").ap()
    sw_d = nc.dram_tensor("scale_w", [2 * KV], dt, kind="ExternalInput").ap()
    sb_d = nc.dram_tensor("scale_b", [1], dt, kind="ExternalInput").ap()
    if WEIGHT_SHARD:
        # this core's 128-column c-slice of each weight + the same slice of
        # every batch's key/value rows
        kw_d = nc.dram_tensor("kw_slice", [D, 128], dt,
                              kind="ExternalInput").ap()
        vw_d = nc.dram_tensor("vw_slice", [D, 128], dt,
                              kind="ExternalInput").ap()
        keysl_d = nc.dram_tensor("key_slice", [B, 128], dt,
                                 kind="ExternalInput").ap()
        valsl_d = nc.dram_tensor("value_slice", [B, 128], dt,
                                 kind="ExternalInput").ap()
        part_in_d = nc.dram_tensor("kv_part_in", [B, 2, D], dt).ap()
        part_out_d = nc.dram_tensor("kv_part_out", [2, D], dt).ap()
    else:
        key_d = nc.dram_tensor("key", [D], dt, kind="ExternalInput").ap()
        val_d = nc.dram_tensor("value", [D], dt, kind="ExternalInput").ap()
        kw_d = nc.dram_tensor("k_weight", [D, D], dt,
                              kind="ExternalInput").ap()
        vw_d = nc.dram_tensor("v_weight", [D, D], dt,
                              kind="ExternalInput").ap()
        kvrow_d = nc.dram_tensor("kvec_row", [D], dt).ap()
        vvrow_d = nc.dram_tensor("vvec_row", [D], dt).ap()

    attn_d = nc.dram_tensor("attn", [H, QL, KV], dt, kind="ExternalOutput").ap()
    out_d = nc.dram_tensor("out", [QL, D], dt, kind="ExternalOutput").ap()

    cr_d = nc.dram_tensor("consts_row", [4], dt).ap()

    def bcast(dram_ap, parts):
        """DRAM AP replicated across `parts` partitions (partition step 0)."""
        return bass.AP(tensor=dram_ap.tensor, offset=dram_ap.offset,
                       ap=[[0, parts]] + list(dram_ap.ap))

    A = mybir.AluOpType

    with tile.TileContext(nc) as tc:
        with (
            tc.tile_pool(name="const", bufs=1) as constp,
            tc.tile_pool(name="wload", bufs=3) as wloadp,
            tc.tile_pool(name="qload", bufs=3) as qloadp,
            tc.tile_pool(name="mid", bufs=3) as midp,
            tc.tile_pool(name="ebuf", bufs=2) as ebufp,
            tc.tile_pool(name="abuf", bufs=3) as abufp,
            tc.tile_pool(name="psum", bufs=1, space="PSUM") as psump,
            tc.tile_pool(name="psumc", bufs=1, space="PSUM") as psumcp,
        ):
          for _full_rep in range(full_unroll):
            # ================= setup (tiny) =================
            ident = constp.tile([128, 128], dt)
            make_identity(nc, ident[:, :])

            clip_row = constp.tile([1, KV], dt)
            nc.sync.dma_start(out=clip_row[:, :],
                              in_=clip_d.rearrange("(a k) -> a k", a=1))
            clipB = constp.tile([128, KV], dt)
            nc.gpsimd.dma_start(out=clipB[:, :], in_=bcast(clip_d, 128))
            # ACT reads from PSUM are cheaper (172 vs 224 cyc fixed)
            clipB_ps = psumcp.tile([128, KV], fp32)
            nc.vector.tensor_copy(clipB_ps[:, :], clipB[:, :])

            if not WEIGHT_SHARD:
                keyB = constp.tile([128, D], dt)
                nc.gpsimd.dma_start(out=keyB[:, :], in_=bcast(key_d, 128))
                valB = constp.tile([128, D], dt)
                nc.gpsimd.dma_start(out=valB[:, :], in_=bcast(val_d, 128))

            sw_row = constp.tile([1, 2 * KV], dt)
            nc.sync.dma_start(out=sw_row[:, :],
                              in_=sw_d.rearrange("(a k) -> a k", a=1))
            sb_t = constp.tile([1, 1], dt)
            nc.sync.dma_start(out=sb_t[:, :],
                              in_=sb_d.rearrange("(a k) -> a k", a=1))

            maxc = constp.tile([1, 1], dt)
            nc.vector.tensor_reduce(maxc[:, :], clip_row[:, :],
                                    axis=mybir.AxisListType.X, op=A.max)
            minc = constp.tile([1, 1], dt)
            nc.vector.tensor_reduce(minc[:, :], clip_row[:, :],
                                    axis=mybir.AxisListType.X, op=A.min)
            cbraw = constp.tile([1, 1], dt)
            scr1 = constp.tile([1, KV], dt)
            nc.vector.tensor_mul(scr1[:, :], clip_row[:, :],
                                 sw_row[:, KV:2 * KV])
            nc.vector.reduce_sum(cbraw[:, :], scr1[:, :],
                                 axis=mybir.AxisListType.X)
            sumw1 = constp.tile([1, 1], dt)
            nc.vector.reduce_sum(sumw1[:, :], sw_row[:, 0:KV],
                                 axis=mybir.AxisListType.X)

            # consts row: [cbtot, -maxclip, -minclip, sumw1/sqrt(HD)]
            cr = constp.tile([1, 4], dt)
            nc.vector.tensor_add(cr[:, 0:1], cbraw[:, :], sb_t[:, :])
            nc.vector.tensor_scalar_mul(cr[:, 1:2], maxc[:, :], -1.0)
            nc.vector.tensor_scalar_mul(cr[:, 2:3], minc[:, :], -1.0)
            nc.vector.tensor_scalar_mul(cr[:, 3:4], sumw1[:, :], 1.0 / 8.0)
            nc.sync.dma_start(out=cr_d.rearrange("(a k) -> a k", a=1),
                              in_=cr[:, :])
            constsB = constp.tile([128, 4], dt)
            nc.gpsimd.dma_start(out=constsB[:, :], in_=bcast(cr_d, 128))
            cbtot_v = constsB[:, 0:1]
            negmaxc_v = constsB[:, 1:2]
            negminc_v = constsB[:, 2:3]
            s18_v = constsB[:, 3:4]

            # ================= Kvec / Vvec =================
            KvecB = constp.tile([128, D], dt)
            VvecB = constp.tile([128, D], dt)
            if WEIGHT_SHARD:
                # W^T slices via PE transpose: [d,c] chunks -> WT [c=128, d]
                wts = []
                for w_ap in (kw_d, vw_d):
                    wt = constp.tile([128, D], dt)
                    for half in range(2):
                        ps_w = psump.tile([128, 512], fp32, tag="psw")
                        for jj in range(4):
                            j = half * 4 + jj
                            wchunk = wloadp.tile([128, 128], dt, tag="wchunk")
                            nc.sync.dma_start(
                                out=wchunk[:, :],
                                in_=w_ap[j * 128:(j + 1) * 128, :])
                            nc.tensor.transpose(
                                ps_w[:, jj * 128:(jj + 1) * 128],
                                wchunk[:, :], ident[:, :])
                        nc.vector.tensor_copy(
                            wt[:, half * 512:(half + 1) * 512], ps_w[:, :])
                    wts.append(wt)

                # keysT/valsT: [8,128] slices -> [128, 8]
                kvt = []
                for sl_d in (keysl_d, valsl_d):
                    sl = wloadp.tile([B, 128], dt, tag="kvsl")
                    nc.sync.dma_start(out=sl[:, :], in_=sl_d)
                    ps_s = psump.tile([128, B], fp32, tag="pss")
                    nc.tensor.transpose(ps_s[:, :], sl[:, :],
                                        ident[0:B, 0:B])
                    slT = constp.tile([128, B], dt)
                    nc.vector.tensor_copy(slT[:, :], ps_s[:, :])
                    kvt.append(slT)

                # partial (all batches): part[b, d] = sum_c key[b,c] W[d,c]
                kv_part = constp.tile([B, 2 * D], dt)
                for wi in range(2):
                    ps_kv = psump.tile([B, D], fp32, tag="pskv")
                    for half in range(2):
                        nc.tensor.matmul(
                            ps_kv[:, half * 512:(half + 1) * 512],
                            lhsT=kvt[wi][:, :],
                            rhs=wts[wi][:, half * 512:(half + 1) * 512],
                            start=True, stop=True)
                    nc.vector.tensor_copy(kv_part[:, wi * D:(wi + 1) * D],
                                          ps_kv[:, :])
                nc.sync.dma_start(
                    out=part_in_d.rearrange("b w d -> b (w d)"),
                    in_=kv_part[:, :])
                nc.gpsimd.collective_compute(
                    "ReduceScatter", A.add,
                    replica_groups=[list(range(N_CORES))],
                    ins=[part_in_d.opt()], outs=[part_out_d.opt()])
                nc.gpsimd.dma_start(out=KvecB[:, :],
                                    in_=bcast(part_out_d[0], 128))
                nc.gpsimd.dma_start(out=VvecB[:, :],
                                    in_=bcast(part_out_d[1], 128))
            else:
                kvec_c = constp.tile([128, 8], dt)   # [d%128, d//128]
                vvec_c = constp.tile([128, 8], dt)
                for w_ap, srcB, dest in ((kw_d, keyB, kvec_c),
                                         (vw_d, valB, vvec_c)):
                    for j in range(8):
                        wchunk = wloadp.tile([128, D], dt, tag="wchunk")
                        nc.sync.dma_start(out=wchunk[:, :],
                                          in_=w_ap[j * 128:(j + 1) * 128, :])
                        wscr = wloadp.tile([128, D], dt, tag="wscr")
                        nc.vector.tensor_mul(wscr[:, :], wchunk[:, :],
                                             srcB[:, :])
                        nc.vector.reduce_sum(dest[:, j:j + 1], wscr[:, :],
                                             axis=mybir.AxisListType.X)
                # transpose [128,8] -> [8,128] rows, bounce via DRAM, bcast
                for src, row_d, dstB in ((kvec_c, kvrow_d, KvecB),
                                         (vvec_c, vvrow_d, VvecB)):
                    ps_t = psump.tile([8, 128], fp32, tag="pst")
                    nc.tensor.transpose(ps_t[:, :], src[:, :], ident[:, :])
                    rows = constp.tile([8, 128], dt)
                    nc.vector.tensor_copy(rows[:, :], ps_t[:, :])
                    nc.sync.dma_start(
                        out=row_d.rearrange("(a k) -> a k", a=8),
                        in_=rows[:, :])
                    nc.gpsimd.dma_start(out=dstB[:, :], in_=bcast(row_d, 128))

            # ================= main loop =================
            def body(_iv=None):
                for qt in range(NQT):
                    q0 = qt * 128
                    qtile = qloadp.tile([128, D], dt, tag="qtile")
                    nc.sync.dma_start(out=qtile[:, :],
                                      in_=q_d[q0:q0 + 128, :])

                    # out rows are just Vvec
                    nc.sync.dma_start(out=out_d[q0:q0 + 128, :],
                                      in_=VvecB[:, :])

                    prod = qloadp.tile([128, D], dt, tag="prod")
                    nc.vector.tensor_mul(prod[:, :], qtile[:, :], KvecB[:, :])
                    lin = midp.tile([128, H], dt, tag="lin")
                    nc.vector.reduce_sum(
                        lin[:, :],
                        prod[:, :].rearrange("p (h d) -> p h d", d=HD),
                        axis=mybir.AxisListType.X)
                    # lin = linraw * (sumw1/sqrt(HD)) + cbtot
                    nc.vector.tensor_scalar(lin[:, :], lin[:, :], s18_v,
                                            cbtot_v, op0=A.mult, op1=A.add)
                    # negm = min(lin*-maxc, lin*-minc) = -max_k(lin*clip)
                    nega = midp.tile([128, H], dt, tag="nega")
                    nc.vector.tensor_scalar_mul(nega[:, :], lin[:, :], negmaxc_v)
                    negm = midp.tile([128, H], dt, tag="negm")
                    nc.vector.tensor_scalar_mul(negm[:, :], lin[:, :], negminc_v)
                    nc.vector.tensor_tensor(negm[:, :], nega[:, :], negm[:, :],
                                            op=A.min)

                    e3 = ebufp.tile([128, H * KV], dt, tag="e3")
                    S = midp.tile([128, H], dt, tag="S")
                    for h in range(H):
                        nc.scalar.activation(
                            e3[:, h * KV:(h + 1) * KV], clipB_ps[:, :],
                            mybir.ActivationFunctionType.Exp,
                            bias=negm[:, h:h + 1], scale=lin[:, h:h + 1],
                            accum_out=S[:, h:h + 1])
                    rinv = midp.tile([128, H], dt, tag="rinv")
                    nc.vector.reciprocal(rinv[:, :], S[:, :])

                    at = abufp.tile([128, H * KV], dt, tag="at")
                    for h in range(H):
                        nc.vector.tensor_scalar_mul(
                            at[:, h * KV:(h + 1) * KV],
                            e3[:, h * KV:(h + 1) * KV], rinv[:, h:h + 1])

                    # one 2 MiB DMA: [128q, 16h, 256k] -> attn[h, q0+q, k]
                    a_sl = attn_d[:, q0:q0 + 128, :]
                    a_perm = bass.AP(tensor=a_sl.tensor, offset=a_sl.offset,
                                     ap=[list(a_sl.ap[1]), list(a_sl.ap[0]),
                                         list(a_sl.ap[2])])
                    nc.sync.dma_start(
                        out=a_perm,
                        in_=at[:, :].rearrange("p (h k) -> p h k", k=KV))

            if repeat == 1:
                body()
            else:
                hint = (mybir.EngineType.Activation, mybir.EngineType.DVE,
                        mybir.EngineType.SP)
                with tc.For_i(0, repeat, 1, hint_engines=hint) as iv:
                    body(iv)

    nc.compile()
    return nc


def _get_nc(repeat: int = 1, full_unroll: int = 1):
    with _lock:
        key = ("nc", repeat, full_unroll)
        if key not in _cache:
            _cache[key] = _build_nc(repeat, full_unroll)
        return _cache[key]


def make_in_maps(query, key, value, clip_score, k_weight, v_weight, scale_w,
                 scale_b):
    query = np.ascontiguousarray(np.asarray(query, dtype=np.float32))
    key = np.ascontiguousarray(np.asarray(key, dtype=np.float32))
    value = np.ascontiguousarray(np.asarray(value, dtype=np.float32))
    clip_score = np.ascontiguousarray(np.asarray(clip_score, dtype=np.float32))
    k_weight = np.ascontiguousarray(np.asarray(k_weight, dtype=np.float32))
    v_weight = np.ascontiguousarray(np.asarray(v_weight, dtype=np.float32))
    scale_w = np.ascontiguousarray(np.asarray(scale_w, dtype=np.float32))
    scale_b = np.asarray(scale_b, dtype=np.float32).reshape(1)

    in_maps = []
    for b in range(B):
        m = {
            "query": query[b],
            "clip_score": clip_score[b],
            "scale_w": scale_w,
            "scale_b": scale_b,
        }
        if WEIGHT_SHARD:
            cs = slice(b * 128, (b + 1) * 128)
            m["kw_slice"] = np.ascontiguousarray(k_weight[:, cs])
            m["vw_slice"] = np.ascontiguousarray(v_weight[:, cs])
            m["key_slice"] = np.ascontiguousarray(key[:, cs])
            m["value_slice"] = np.ascontiguousarray(value[:, cs])
        else:
            m["key"] = key[b]
            m["value"] = value[b]
            m["k_weight"] = k_weight
            m["v_weight"] = v_weight
        in_maps.append(m)
    return in_maps


def kernel(query, key, value, clip_score, k_weight, v_weight, scale_w,
           scale_b):
    from concourse.bass_utils import run_bass_kernel_spmd

    nc = _get_nc()
    in_maps = make_in_maps(query, key, value, clip_score, k_weight, v_weight,
                           scale_w, scale_b)
    res = run_bass_kernel_spmd(nc, in_maps, core_ids=list(range(N_CORES)))
    attn = np.stack([res.results[b]["attn"] for b in range(B)])
    out = np.stack([res.results[b]["out"] for b in range(B)])
    return out, attn


if __name__ == "__main__":
    # quick smoke: random inputs, shape check only
    rng = np.random.default_rng(0)
    o, a = kernel(
        rng.standard_normal((B, QL, D), dtype=np.float32),
        rng.standard_normal((B, D), dtype=np.float32),
        rng.standard_normal((B, D), dtype=np.float32),
        rng.random((B, KV), dtype=np.float32),
        (rng.standard_normal((D, D), dtype=np.float32) * 0.02),
        (rng.standard_normal((D, D), dtype=np.float32) * 0.02),
        np.full((2 * KV,), 0.05, dtype=np.float32),
        np.float32(7.5),
    )
    print(o.shape, a.shape)
